# revision 26
# baseline (speedup 1.0000x reference)
"""Trainium2 Bass kernel: causal multi-head attention block (B=2, S=2048, D=2048, H=16).

Sharding: 8 cores = 2 (batch) x 4 (head-groups of 4 heads). Each core computes
its batch's attention output restricted to its 4 heads plus the corresponding
partial out-projection; the host sums the 4 head-group partials per batch and
adds the (o_b + o_w @ v_b) bias vector (valid because softmax rows sum to 1).

v5: all matmul operands in bf16. Inputs are host-retiled to [128, n, cols]
layouts so each weight/x block lands in one DMA descriptor (the Sync engine
issues descriptors serially at ~650ns each), chunked so the first matmul's
dependencies arrive first. x is read once per 512-column block, shared by the
Q/K and V projection passes. Q^T/K^T live in per-(head, block) tiles so
attention on block J depends only on block J's projections, letting the
emission fully interleave per block: A0 B0 A1 B1 C0 A2 B2 C1 A3 B3 C2 C3.
Softmax normalization is a reciprocal on the [1, q] row + partition broadcast
(gpsimd); for the last block's final head the broadcast runs as a K=1
ones-matmul on the then-idle PE instead, shortening the only unhidden chain.
The out-projection reuses each stationary tile for 4 consecutive matmuls and
stages a full [128, 2048] output row block per DMA.
"""

import sys

sys.path.insert(0, "/opt/trn_rl_repo")

import numpy as np
import ml_dtypes
import concourse.bacc as bacc
import concourse.tile as tile
from concourse import mybir
from concourse.bass_utils import run_bass_kernel_spmd

F32 = mybir.dt.float32
F32R = mybir.dt.float32r
BF16 = mybir.dt.bfloat16
BF = ml_dtypes.bfloat16

B, S, D, H, HD = 2, 2048, 2048, 16, 128
SCALE = 1.0 / (HD**0.5)
HL = 4  # heads per core
DL = HL * HD  # 512: local head dims per core
NK = D // HD  # 16 contraction k-tiles
NJ = S // DL  # 4 blocks of 512 along sequence
NST = S // HD  # 16 output row tiles
NEG = -1.0e30

_CACHE = {}


def _build():
    nc = bacc.Bacc("TRN2", target_bir_lowering=False, debug=False)
    ExpF = mybir.ActivationFunctionType.Exp

    x_d = [
        nc.declare_dram_parameter("x0a", [HD, 4, DL], BF16, isOutput=False),
        nc.declare_dram_parameter("x0b", [HD, 4, DL], BF16, isOutput=False),
        nc.declare_dram_parameter("x0c", [HD, 8, DL], BF16, isOutput=False),
        nc.declare_dram_parameter("x1", [HD, NK, DL], BF16, isOutput=False),
        nc.declare_dram_parameter("x2", [HD, NK, DL], BF16, isOutput=False),
        nc.declare_dram_parameter("x3", [HD, NK, DL], BF16, isOutput=False),
    ]
    wq_d = [
        nc.declare_dram_parameter("wqa", [HD, 4, DL], BF16, isOutput=False),
        nc.declare_dram_parameter("wqb", [HD, 6, DL], BF16, isOutput=False),
        nc.declare_dram_parameter("wqc", [HD, 6, DL], BF16, isOutput=False),
    ]
    wk_d = [
        nc.declare_dram_parameter("wka", [HD, 4, DL], BF16, isOutput=False),
        nc.declare_dram_parameter("wkb", [HD, 6, DL], BF16, isOutput=False),
        nc.declare_dram_parameter("wkc", [HD, 6, DL], BF16, isOutput=False),
    ]
    wv3 = nc.declare_dram_parameter("wv3", [HD, NK, DL], BF16, isOutput=False)
    wo3 = nc.declare_dram_parameter("wo3", [HD, HL, D], BF16, isOutput=False)
    bq = nc.declare_dram_parameter("bq", [HD, HL], F32, isOutput=False)
    bk = nc.declare_dram_parameter("bk", [HD, HL], F32, isOutput=False)
    mask3 = nc.declare_dram_parameter("mask3", [HD, 4, DL], BF16, isOutput=False)
    onec = nc.declare_dram_parameter("onec", [HD, 1], BF16, isOutput=False)
    oner = nc.declare_dram_parameter("oner", [1, HD], F32, isOutput=False)
    out = nc.declare_dram_parameter("out", [HD, NST, D], BF16, isOutput=True)

    with tile.TileContext(nc) as tc:
        with (
            tc.tile_pool(name="const", bufs=1) as constp,
            tc.tile_pool(name="qk", bufs=1) as qkp,
            tc.tile_pool(name="vres", bufs=1) as vp,
            tc.tile_pool(name="xs", bufs=2) as xs,
            tc.tile_pool(name="x0p", bufs=1) as x0p,
            tc.tile_pool(name="wts", bufs=1) as wts,
            tc.tile_pool(name="pt", bufs=6) as ptp,
            tc.tile_pool(name="attn", bufs=8) as attnp,
            tc.tile_pool(name="norm", bufs=3) as normp,
            tc.tile_pool(name="ob", bufs=3) as obp,
            tc.tile_pool(name="ps", bufs=8, space="PSUM") as ps,
        ):
            # --- residents: per-(head, block) Q^T/K^T so attention on block J
            # only depends on block J's projection evictions ---
            QT = [[qkp.tile([HD, DL], BF16, tag=f"qt{h}_{J}", name=f"qt{h}_{J}") for J in range(NJ)] for h in range(HL)]
            KT = [[qkp.tile([HD, DL], BF16, tag=f"kt{h}_{J}", name=f"kt{h}_{J}") for J in range(NJ)] for h in range(HL)]
            V = [vp.tile([HD, DL], BF16, tag=f"v{t}", name=f"v{t}") for t in range(NST)]

            # --- DMA emission order == consumption order (Sync issues
            # descriptors serially): weight/x chunks for the first matmuls
            # first, then the rest ---
            wq_c, wk_c = [], []
            CH = [(0, 4), (4, 10), (10, NK)]
            x0 = []

            def chunk(lst, k):
                for (lo, hi), t in zip(CH, lst):
                    if k < hi:
                        return t[:, k - lo, :]

            wq_c.append(wts.tile([HD, 4, DL], BF16, tag="wqa", name="wqa"))
            nc.sync.dma_start(wq_c[0][:], wq_d[0][:, :, :])
            x0.append(x0p.tile([HD, 4, DL], BF16, tag="x0a", name="x0a"))
            nc.sync.dma_start(x0[0][:], x_d[0][:, :, :])
            wk_c.append(wts.tile([HD, 4, DL], BF16, tag="wka", name="wka"))
            nc.sync.dma_start(wk_c[0][:], wk_d[0][:, :, :])
            x0.append(x0p.tile([HD, 4, DL], BF16, tag="x0b", name="x0b"))
            nc.sync.dma_start(x0[1][:], x_d[1][:, :, :])
            wq_c.append(wts.tile([HD, 6, DL], BF16, tag="wqb", name="wqb"))
            nc.sync.dma_start(wq_c[1][:], wq_d[1][:, :, :])
            wk_c.append(wts.tile([HD, 6, DL], BF16, tag="wkb", name="wkb"))
            nc.sync.dma_start(wk_c[1][:], wk_d[1][:, :, :])
            x0.append(x0p.tile([HD, 8, DL], BF16, tag="x0c", name="x0c"))
            nc.sync.dma_start(x0[2][:], x_d[2][:, :, :])
            wq_c.append(wts.tile([HD, 6, DL], BF16, tag="wqc", name="wqc"))
            nc.sync.dma_start(wq_c[2][:], wq_d[2][:, :, :])
            wk_c.append(wts.tile([HD, 6, DL], BF16, tag="wkc", name="wkc"))
            nc.sync.dma_start(wk_c[2][:], wk_d[2][:, :, :])
            bq_sb = constp.tile([HD, HL], F32, tag="bq")
            nc.sync.dma_start(bq_sb[:], bq[:, :])
            bk_sb = constp.tile([HD, HL], F32, tag="bk")
            nc.sync.dma_start(bk_sb[:], bk[:, :])
            onec_sb = constp.tile([HD, 1], BF16, tag="onec")
            nc.sync.dma_start(onec_sb[:], onec[:, :])
            oner_sb = constp.tile([1, HD], F32, tag="oner")
            nc.sync.dma_start(oner_sb[:], oner[:, :])
            mask_sb = constp.tile([HD, 4, DL], BF16, tag="mask")
            nc.sync.dma_start(mask_sb[:], mask3[:, :, :])
            wv_sb = wts.tile([HD, NK, DL], BF16, tag="wv")
            nc.sync.dma_start(wv_sb[:], wv3[:, :, :])
            wo_sb = constp.tile([HD, HL, D], BF16, tag="wo")
            nc.sync.dma_start(wo_sb[:], wo3[:, :, :])

            attn_all = {}

            xtiles = {}

            def get_x(J):
                if J in xtiles:
                    return xtiles[J]
                if J == 0:
                    def xsl(k):
                        return x0[0][:, k, :] if k < 4 else (
                            x0[1][:, k - 4, :] if k < 8 else x0[2][:, k - 8, :])
                else:
                    xj = xs.tile([HD, NK, DL], BF16, tag="xt", name=f"x{J}")
                    nc.sync.dma_start(xj[:], x_d[J + 2][:, :, :])

                    def xsl(k):
                        return xj[:, k, :]
                xtiles[J] = xsl
                return xsl

            def phase_a(J):
                xsl = get_x(J)
                # A1: Q^T and K^T for this 512-wide s-block, all 4 heads
                qps = [ps.tile([HD, DL], F32, tag="ps", name=f"qps{J}_{i}") for i in range(HL)]
                kps = [ps.tile([HD, DL], F32, tag="ps", name=f"kps{J}_{i}") for i in range(HL)]
                for k in range(NK):
                    for h in range(HL):
                        sl_h = slice(HD * h, HD * (h + 1))
                        nc.tensor.matmul(
                            qps[h][:], chunk(wq_c, k)[:, sl_h], xsl(k),
                            start=(k == 0), stop=(k == NK - 1),
                        )
                        nc.tensor.matmul(
                            kps[h][:], chunk(wk_c, k)[:, sl_h], xsl(k),
                            start=(k == 0), stop=(k == NK - 1),
                        )
                for h in range(HL):
                    nc.scalar.add(QT[h][J][:], qps[h][:], bq_sb[:, h : h + 1])
                    with nc.allow_low_precision(reason="bf16 rounding only"):
                        nc.vector.tensor_scalar_add(KT[h][J][:], kps[h][:], bk_sb[:, h : h + 1])
                # A2: V natural layout for the 4 s-tiles of this block
                vps = [ps.tile([HD, DL], F32, tag="ps", name=f"vps{J}_{i}") for i in range(4)]
                for k in range(NK):
                    for t in range(4):
                        nc.tensor.matmul(
                            vps[t][:],
                            xsl(k)[:, HD * t : HD * (t + 1)],
                            wv_sb[:, k, :],
                            start=(k == 0), stop=(k == NK - 1),
                        )
                for t in range(4):
                    with nc.allow_low_precision(reason="bf16 rounding only"):
                        nc.vector.tensor_copy(V[4 * J + t][:], vps[t][:])

            def phase_b(J, heads=range(HL)):
                attn_t = attn_all.setdefault(J, [])
                for h in heads:
                    sl_h = slice(HD * h, HD * (h + 1))
                    nkt = 4 * (J + 1)  # causal: key tiles 0..nkt-1
                    aps = ps.tile([HD, DL], F32, tag="ps")
                    sps = ps.tile([HD, DL], F32, tag="ps")
                    for i in range(nkt):
                        # causal column shrink: diagonal tile r only touches
                        # q-columns >= 128*r within this block
                        r = i - 4 * J
                        qlo = HD * r if r >= 0 else 0
                        cs = slice(qlo, DL)
                        scp = ps.tile([HD, DL], F32, tag="ps")
                        nc.tensor.matmul(
                            scp[:, cs],
                            KT[h][i // 4][:, HD * (i % 4) : HD * (i % 4 + 1)],
                            QT[h][J][:, cs],
                            start=True, stop=True,
                        )
                        if r >= 0:
                            # mask is nonzero only in the 128-wide diagonal
                            # block [128r, 128(r+1))
                            ms = slice(qlo, HD * (r + 1))
                            nc.vector.tensor_add(scp[:, ms], scp[:, ms], mask_sb[:, r, ms])
                        ptt = ptp.tile([HD, DL], BF16, tag="pt")
                        nc.scalar.activation(ptt[:, cs], scp[:, cs], ExpF)
                        nc.tensor.matmul(
                            aps[:, cs], V[i][:, sl_h], ptt[:, cs],
                            start=(i == 0), stop=(i == nkt - 1),
                        )
                        nc.tensor.matmul(
                            sps[0:1, cs], onec_sb[:], ptt[:, cs],
                            start=(i == 0), stop=(i == nkt - 1),
                        )
                    bcs = normp.tile([HD, DL], F32, tag="bc")
                    if J == NJ - 1 and h == HL - 1:
                        # last chain before C(J3) is unhidden: broadcast on the
                        # then-idle PE (K=1 ones matmul) + scalar evict instead
                        # of the higher-latency gpsimd path
                        rrr = constp.tile([1, DL], F32, tag="rrr")
                        nc.vector.reciprocal_approx_fast(rrr[:], sps[0:1, :])
                        bcp = ps.tile([HD, DL], F32, tag="ps")
                        nc.tensor.matmul(bcp[:], oner_sb[:], rrr[:], start=True, stop=True)
                        nc.scalar.copy(bcs[:], bcp[:])
                    else:
                        rr = normp.tile([1, DL], F32, tag="rr")
                        nc.vector.reciprocal_approx_fast(rr[:], sps[0:1, :])
                        nc.gpsimd.partition_broadcast(bcs[:], rr[:])
                    at = attnp.tile([HD, DL], BF16, tag="at")
                    with nc.allow_low_precision(reason="bf16 rounding only"):
                        nc.vector.tensor_mul(at[:], aps[:], bcs[:])
                    attn_t.append(at)

            def phase_c(J):
                attn_t = attn_all.pop(J)
                for c in range(4):
                    sl_c = slice(HD * c, HD * (c + 1))
                    st = 4 * J + c
                    ob = obp.tile([HD, D], BF16, tag="ob")
                    ops = [ps.tile([HD, DL], F32, tag="ps", name=f"op{st}_{nb}") for nb in range(4)]
                    for dh in range(HL):
                        for nb in range(4):
                            sl_n = slice(DL * nb, DL * (nb + 1))
                            nc.tensor.matmul(
                                ops[nb][:], attn_t[dh][:, sl_c], wo_sb[:, dh, sl_n],
                                start=(dh == 0), stop=(dh == HL - 1),
                            )
                    if J == NJ - 1 and c == 3:
                        # kernel tail: spread the final evictions over both
                        # engines and split the DMA so transfer overlaps them
                        for nb in range(4):
                            sl_n = slice(DL * nb, DL * (nb + 1))
                            if nb % 2 == 0:
                                nc.scalar.copy(ob[:, sl_n], ops[nb][:])
                            else:
                                with nc.allow_low_precision(reason="bf16 rounding only"):
                                    nc.vector.tensor_copy(ob[:, sl_n], ops[nb][:])
                            if nb == 1:
                                nc.sync.dma_start(out[:, st, 0 : 2 * DL], ob[:, 0 : 2 * DL])
                        nc.sync.dma_start(out[:, st, 2 * DL :], ob[:, 2 * DL :])
                    else:
                        for nb in range(4):
                            sl_n = slice(DL * nb, DL * (nb + 1))
                            with nc.allow_low_precision(reason="bf16 rounding only"):
                                nc.vector.tensor_copy(ob[:, sl_n], ops[nb][:])
                        nc.sync.dma_start(out[:, st, :], ob[:])

            phase_a(0)
            phase_a(1)
            phase_b(0)
            phase_a(2)
            phase_b(1)
            phase_c(0)
            phase_a(3)
            phase_b(2)
            phase_c(1)
            phase_b(3, range(0, 3))
            phase_c(2)
            phase_b(3, range(3, 4))
            phase_c(3)

    nc.compile()
    return nc


def _prep_in_maps(x, q_w, q_b, k_w, k_b, v_w, v_b, o_w, o_b):
    maskT = np.where(
        np.arange(DL)[:, None] > np.arange(DL)[None, :], np.float32(NEG), np.float32(0)
    ).astype(np.float32)
    mask3 = np.ascontiguousarray(maskT.reshape(4, HD, DL).transpose(1, 0, 2)).astype(BF)

    def tile3(w):  # [D, C] -> [HD, D//HD, C]
        d, c = w.shape
        return np.ascontiguousarray(w.reshape(d // HD, HD, c).transpose(1, 0, 2))

    onec = np.ones((HD, 1), BF)
    oner = np.ones((1, HD), np.float32)
    in_maps = []
    for cid in range(8):
        b, hg = divmod(cid, 4)
        ds = slice(DL * hg, DL * (hg + 1))
        xt = tile3(np.ascontiguousarray(x[b].T)).astype(BF)
        wqt = tile3(q_w[ds].T * SCALE).astype(BF)
        wkt = tile3(np.ascontiguousarray(k_w[ds].T)).astype(BF)
        in_maps.append(
            {
                "x0a": np.ascontiguousarray(xt[:, 0:4, 0:DL]),
                "x0b": np.ascontiguousarray(xt[:, 4:8, 0:DL]),
                "x0c": np.ascontiguousarray(xt[:, 8:NK, 0:DL]),
                "x1": np.ascontiguousarray(xt[:, :, DL:2 * DL]),
                "x2": np.ascontiguousarray(xt[:, :, 2 * DL:3 * DL]),
                "x3": np.ascontiguousarray(xt[:, :, 3 * DL:4 * DL]),
                "wqa": np.ascontiguousarray(wqt[:, 0:4, :]),
                "wqb": np.ascontiguousarray(wqt[:, 4:10, :]),
                "wqc": np.ascontiguousarray(wqt[:, 10:NK, :]),
                "wka": np.ascontiguousarray(wkt[:, 0:4, :]),
                "wkb": np.ascontiguousarray(wkt[:, 4:10, :]),
                "wkc": np.ascontiguousarray(wkt[:, 10:NK, :]),
                "wv3": tile3(np.ascontiguousarray(v_w[ds].T)).astype(BF),
                "wo3": tile3(np.ascontiguousarray(o_w[:, ds].T)).astype(BF),
                "bq": np.ascontiguousarray((q_b[ds] * SCALE).reshape(HL, HD).T),
                "bk": np.ascontiguousarray(k_b[ds].reshape(HL, HD).T),
                "mask3": mask3,
                "onec": onec,
                "oner": oner,
            }
        )
    return in_maps


def kernel(x, q_w, q_b, k_w, k_b, v_w, v_b, o_w, o_b, _trace=False, _trace_kwargs=None):
    x = np.asarray(x, np.float32)
    args = [np.asarray(a, np.float32) for a in (q_w, q_b, k_w, k_b, v_w, v_b, o_w, o_b)]
    q_w, q_b, k_w, k_b, v_w, v_b, o_w, o_b = args

    if "nc" not in _CACHE:
        _CACHE["nc"] = _build()
    nc = _CACHE["nc"]

    in_maps = _prep_in_maps(x, q_w, q_b, k_w, k_b, v_w, v_b, o_w, o_b)
    res = run_bass_kernel_spmd(
        nc, in_maps, list(range(8)), trace=_trace, **(_trace_kwargs or {})
    )
    _CACHE["last_result"] = res

    bias_vec = (o_w @ v_b + o_b).astype(np.float32)
    out = np.empty((B, S, D), np.float32)
    for b in range(B):
        acc = res.results[4 * b]["out"].astype(np.float32).transpose(1, 0, 2).reshape(S, D).copy()
        for hg in range(1, 4):
            acc += res.results[4 * b + hg]["out"].astype(np.float32).transpose(1, 0, 2).reshape(S, D)
        out[b] = acc + bias_vec
    return out
